# revision 1
# baseline (speedup 1.0000x reference)
"""KANConv2D Trainium2 kernel (8 NeuronCores, data-parallel over batch).

Math: out = conv(x, kernel) + exp(-gamma * d) + bias, where
  d[n,f]  = pn[n] + cn[f] - 2*pc[n,f]
  pc      = conv(x, control_points)      (patches @ control_points)
  pn[n]   = sum of x^2 over the 3x3xC patch
  gamma   = 1 / (2 * mean(d))            (global mean -> AllReduce)

Device strategy per core (4 images):
  - host pre-transposes/pads the shard to [C=64, img, 66, 66] and stacks x^2
    on SBUF partitions 64..127, so q := pc - pn/2 falls out of ONE 9-tap
    accumulated matmul group per 512-pixel block:
      lhsT_tap = [[cp_tap (64x128)], [-1/2 (64x128)]]  (K=128)
  - conv runs as its own 9-tap group (K=64); its results drain to a DRAM
    scratch so the PE keeps working through the AllReduce latency window.
  - matmuls are emitted tap-outer over groups of 3 PSUM blocks and the
    walrus ldw-elision pass is enabled, so one LDWEIGHTS serves 3 matmuls.
  - matmuls use float32r (1-pass FP22 multiply, fp32 PSUM accumulate).
  - Sum(q) per core -> AllReduce -> gamma -> exp epilogue on ACT engine.
"""

import os
import sys

import numpy as np

for _p in ("/opt/trn_rl_repo", "/root/.axon_site/_ro/trn_rl_repo"):
    if os.path.isdir(_p) and _p not in sys.path:
        sys.path.insert(0, _p)

import concourse.bacc as bacc
import concourse.bass_utils as _bu
import concourse.tile as tile
from concourse import mybir
from concourse.bass_utils import run_bass_kernel_spmd


def _ensure_ntff_hook():
    """bass_utils imports antenv.axon_hooks when tracing under axon; this
    image's antenv lacks that module. Provide it and install the ctypes
    NTFF hook so BASS_TRACE=1 yields exec_time_ns."""
    import types
    try:
        from antenv.axon_hooks import get_axon_ntff_profile_hook  # noqa: F401
        return
    except ImportError:
        pass
    try:
        import antenv
        mod = types.ModuleType("antenv.axon_hooks")
        _state = {"hook": None}
        mod.set_axon_ntff_profile_hook = lambda h: _state.__setitem__("hook", h)
        mod.get_axon_ntff_profile_hook = lambda: _state["hook"]
        sys.modules["antenv.axon_hooks"] = mod
        antenv.axon_hooks = mod
        try:
            from trn_agent_boot.trn_boot import _ntff_profile_via_ctypes
            so = "/opt/axon/libaxon_pjrt.so"
            if os.path.exists(so):
                mod.set_axon_ntff_profile_hook(_ntff_profile_via_ctypes(so))
        except Exception:
            pass
    except Exception:
        pass


def _enable_ldw_opt():
    """Consecutive matmuls sharing one weight tile only pay a single
    LDWEIGHTS if walrus's ldw-elision pass runs; concourse pins it off."""
    if getattr(_bu.run_command, "_ldw_patched", False):
        return
    orig = _bu.run_command

    def patched(argv, **kw):
        argv = ["--enable-ldw-opt=true" if a == "--enable-ldw-opt=false" else a
                for a in argv]
        return orig(argv, **kw)

    patched._ldw_patched = True
    _bu.run_command = patched


_ensure_ntff_hook()
_enable_ldw_opt()

B, H, W, C, F = 32, 64, 64, 64, 128
KH = KW = 3
N_CORES = 8
IMGS = B // N_CORES          # 4 images per core
HP, WP = H + 2, W + 2        # 66 padded
ROWS_PER_BLK = 8
BLK = ROWS_PER_BLK * W       # 512 pixels per block
BLKS_PER_IMG = H // ROWS_PER_BLK    # 8
NBLK = IMGS * BLKS_PER_IMG   # 32 blocks per core
PIX = IMGS * H * W           # 16384 pixels per core
NTOT = B * H * W             # 131072 pixels total

F32 = mybir.dt.float32
F32R = mybir.dt.float32r

TAPS = [(kh, kw) for kh in range(KH) for kw in range(KW)]
GROUPS = [(0, 1, 2), (3, 4, 5), (6, 7)]   # hb groups within an image

LAST_EXEC_TIME_NS = None


def _build(offset_const: float, scale_const: float):
    """offset_const = 2*sum(cn)/F ; scale_const = -4/(NTOT*F).
    gamma = 1 / (offset_const + scale_const * sum_q_total)."""
    nc = bacc.Bacc("TRN2", target_bir_lowering=False, debug=False,
                   num_devices=N_CORES)
    xx = nc.dram_tensor("xx", [128, IMGS, HP, WP], F32R, kind="ExternalInput")
    convw = nc.dram_tensor("convw", [64, 9 * F], F32R, kind="ExternalInput")
    qw = nc.dram_tensor("qw", [128, 9 * F], F32R, kind="ExternalInput")
    cnneg = nc.dram_tensor("cnneg", [128, 1], F32, kind="ExternalInput")
    biasf = nc.dram_tensor("biasf", [128, 1], F32, kind="ExternalInput")
    out = nc.dram_tensor("out", [128, PIX], F32, kind="ExternalOutput")

    with tile.TileContext(nc) as tc:
        with (
            tc.tile_pool(name="xp", bufs=1) as xp,
            tc.tile_pool(name="wp", bufs=1) as wp,
            tc.tile_pool(name="qs", bufs=1) as qs,
            tc.tile_pool(name="st", bufs=3) as st,
            tc.tile_pool(name="cs", bufs=6) as cs,
            tc.tile_pool(name="ps", bufs=6, space="PSUM") as ps,
            tc.tile_pool(name="pss", bufs=1, space="PSUM") as pss,
            tc.tile_pool(name="dr", bufs=1, space="DRAM") as dr,
        ):
            # ---- loads ----
            x_t = []
            for i in range(IMGS):
                t = xp.tile([128, HP, WP], F32R, tag=f"x{i}")
                nc.sync.dma_start(out=t, in_=xx[:, i])
                x_t.append(t)
            cw = wp.tile([64, 9 * F], F32R, tag="cw")
            nc.sync.dma_start(out=cw, in_=convw[:])
            qwt = wp.tile([128, 9 * F], F32R, tag="qw")
            nc.sync.dma_start(out=qwt, in_=qw[:])
            cnn = wp.tile([128, 1], F32, tag="cnn")
            nc.sync.dma_start(out=cnn, in_=cnneg[:])
            bft = wp.tile([128, 1], F32, tag="bf")
            nc.sync.dma_start(out=bft, in_=biasf[:])
            ones_c = wp.tile([128, 1], F32, tag="oc")
            nc.vector.memset(ones_c, 1.0)
            ones_r = wp.tile([1, 128], F32, tag="or")
            nc.vector.memset(ones_r, 1.0)

            qst = qs.tile([128, NBLK, BLK], F32, tag="q")
            sq_slots = wp.tile([128, NBLK], F32, tag="sq")

            # ---- phase A: q = pc - pn/2, tap-outer over groups of blocks ----
            for img in range(IMGS):
                xt = x_t[img]
                for grp in GROUPS:
                    qps = [ps.tile([128, BLK], F32, tag="mm", name=f"qp{img}_{hbx}") for hbx in grp]
                    for t, (kh, kw) in enumerate(TAPS):
                        wtile = qwt[:, t * F:(t + 1) * F]
                        for gi, hb in enumerate(grp):
                            h0 = hb * ROWS_PER_BLK
                            rhs = xt[:, h0 + kh:h0 + kh + ROWS_PER_BLK,
                                     kw:kw + W]
                            nc.tensor.matmul(qps[gi][:], wtile, rhs,
                                             start=(t == 0), stop=(t == 8))
                    for gi, hb in enumerate(grp):
                        blk = img * BLKS_PER_IMG + hb
                        nc.scalar.activation(
                            qst[:, blk, :], qps[gi][:],
                            mybir.ActivationFunctionType.Copy,
                            accum_out=sq_slots[:, blk:blk + 1],
                        )

            # ---- gamma: local reduce -> AllReduce -> 1/(off + scale*S) ----
            sq_red = wp.tile([128, 1], F32, tag="sqr")
            nc.vector.reduce_sum(sq_red, sq_slots[:], axis=mybir.AxisListType.X)
            ps1 = pss.tile([1, 1], F32, tag="s1")
            nc.tensor.matmul(ps1[:], sq_red[:], ones_c[:],
                             start=True, stop=True)
            s_sb = wp.tile([1, 1], F32, tag="ssb")
            nc.scalar.copy(s_sb[:], ps1[:])
            cc_in = dr.tile([1, 1], F32, tag="cci")
            cc_out = dr.tile([1, 1], F32, tag="cco")
            nc.sync.dma_start(out=cc_in, in_=s_sb[:])
            nc.gpsimd.collective_compute(
                "AllReduce", mybir.AluOpType.add,
                replica_groups=[list(range(N_CORES))],
                ins=[cc_in.opt()], outs=[cc_out.opt()],
            )
            stot = wp.tile([1, 1], F32, tag="stot")
            nc.sync.dma_start(out=stot, in_=cc_out)
            den = wp.tile([1, 1], F32, tag="den")
            nc.vector.tensor_scalar(
                out=den[:], in0=stot[:],
                scalar1=float(scale_const), scalar2=float(offset_const),
                op0=mybir.AluOpType.mult, op1=mybir.AluOpType.add,
            )
            gam = wp.tile([1, 1], F32, tag="gam")
            nc.vector.reciprocal(gam[:], den[:])
            psg = pss.tile([128, 1], F32, tag="pg")
            nc.tensor.matmul(psg[:], ones_r[:], gam[:],
                             start=True, stop=True)
            scal = wp.tile([128, 1], F32, tag="scal")
            nc.vector.tensor_scalar_mul(scal[:], psg[:], 2.0)
            bias_g = wp.tile([128, 1], F32, tag="bg")
            nc.vector.tensor_mul(bias_g[:], psg[:], cnn[:])

            # ---- phase C: conv, drains to DRAM scratch (no gamma dep, so
            # the PE keeps running during the AllReduce) ----
            cdram = [dr.tile([128, H * W], F32, tag=f"cd{i}", name=f"cd{i}")
                     for i in range(IMGS)]
            for img in range(IMGS):
                xt = x_t[img]
                for grp in GROUPS:
                    cps = [ps.tile([128, BLK], F32, tag="mm", name=f"cp{img}_{hbx}") for hbx in grp]
                    for t, (kh, kw) in enumerate(TAPS):
                        wtile = cw[:, t * F:(t + 1) * F]
                        for gi, hb in enumerate(grp):
                            h0 = hb * ROWS_PER_BLK
                            rhs = xt[0:64, h0 + kh:h0 + kh + ROWS_PER_BLK,
                                     kw:kw + W]
                            nc.tensor.matmul(cps[gi][:], wtile, rhs,
                                             start=(t == 0), stop=(t == 8))
                    for gi, hb in enumerate(grp):
                        cst = cs.tile([128, BLK], F32, tag="cst")
                        nc.vector.tensor_copy(cst[:], cps[gi][:])
                        nc.sync.dma_start(
                            out=cdram[img][:, hb * BLK:(hb + 1) * BLK],
                            in_=cst[:])

            # ---- phase D: epilogue out = conv + exp(2g*q - g*cn) + bias ----
            for img in range(IMGS):
                for hb in range(BLKS_PER_IMG):
                    blk = img * BLKS_PER_IMG + hb
                    ct = st.tile([128, BLK], F32, tag="ct")
                    nc.sync.dma_start(
                        out=ct[:],
                        in_=cdram[img][:, hb * BLK:(hb + 1) * BLK])
                    kan = st.tile([128, BLK], F32, tag="kan")
                    nc.scalar.activation(
                        kan[:], qst[:, blk, :],
                        mybir.ActivationFunctionType.Exp,
                        bias=bias_g[:], scale=scal[:],
                    )
                    outt = st.tile([128, BLK], F32, tag="outt")
                    nc.vector.scalar_tensor_tensor(
                        out=outt[:], in0=kan[:], scalar=bft[:], in1=ct[:],
                        op0=mybir.AluOpType.add, op1=mybir.AluOpType.add,
                    )
                    nc.sync.dma_start(out=out[:, blk * BLK:(blk + 1) * BLK],
                                      in_=outt[:])

    nc.compile()
    return nc


def kernel(inputs, kernel, bias, control_points):
    global LAST_EXEC_TIME_NS
    x = np.ascontiguousarray(np.asarray(inputs, dtype=np.float32))
    kw_ = np.asarray(kernel, dtype=np.float32)
    bias = np.asarray(bias, dtype=np.float32)
    cp = np.asarray(control_points, dtype=np.float32)

    # weights: [kh,kw,C,F] -> [C, (kh*3+kw)*F + f]
    convw = np.ascontiguousarray(
        kw_.transpose(2, 0, 1, 3).reshape(C, 9 * F))
    cpw = cp.transpose(2, 0, 1, 3).reshape(C, 9 * F)
    qw = np.ascontiguousarray(
        np.concatenate([cpw, np.full((C, 9 * F), -0.5, np.float32)], axis=0))

    cn = (cp.reshape(KH * KW * C, F).astype(np.float64) ** 2).sum(axis=0)
    offset_const = float(2.0 * cn.sum() / F)
    scale_const = float(-4.0 / (NTOT * F))
    cnneg = np.ascontiguousarray(-cn.astype(np.float32).reshape(F, 1))
    biasf = np.ascontiguousarray(bias.reshape(F, 1))

    in_maps = []
    for core in range(N_CORES):
        xs = x[core * IMGS:(core + 1) * IMGS]          # [4,64,64,64]
        xt = xs.transpose(3, 0, 1, 2)                  # [C,4,64,64]
        xpad = np.zeros((C, IMGS, HP, WP), np.float32)
        xpad[:, :, 1:H + 1, 1:W + 1] = xt
        xxc = np.ascontiguousarray(
            np.concatenate([xpad, xpad * xpad], axis=0))  # [128,4,66,66]
        in_maps.append({
            "xx": xxc, "convw": convw, "qw": qw,
            "cnneg": cnneg, "biasf": biasf,
        })

    nc = _build(offset_const, scale_const)
    res = run_bass_kernel_spmd(nc, in_maps, core_ids=list(range(N_CORES)))
    LAST_EXEC_TIME_NS = res.exec_time_ns

    out = np.empty((B, H, W, F), np.float32)
    for core in range(N_CORES):
        o = res.results[core]["out"]                   # [128, PIX]
        o = o.reshape(F, IMGS, H, W).transpose(1, 2, 3, 0)
        out[core * IMGS:(core + 1) * IMGS] = o
    return out



# revision 18
# speedup vs baseline: 2.0805x; 2.0805x over previous
"""KANConv2D Trainium2 kernel (8 NeuronCores, data-parallel over batch).

Math: out = conv(x, kernel) + exp(-gamma * d) + bias, where
  d[n,f]  = pn[n] + cn[f] - 2*pc[n,f]
  pc      = patches(x) @ control_points
  pn[n]   = sum of x^2 over the 3x3xC patch
  gamma   = 1 / (2 * mean(d))            (global mean -> AllReduce)

Device strategy per core (4 images), v2:
  - q := pc - pn/2 runs in fp8e4m3 with DoubleRow matmuls: rhs partitions
    hold [x8; x8^2] stored as THREE column-shifted copies with row stride
    exactly 64, so a block's 8x64 window is one contiguous 512-run and the
    DoubleRow ifmap is the required 3-dim [128, 2, 512] AP whose pair dim
    strides between copies/rows (both multiples of 16). Each DoubleRow
    covers two of the 9 taps -> 5 matmuls per 512-pixel block.
  - conv runs in bf16 with K=128 tap pairing: SBUF tile xc = [x | x
    shifted left one column], so taps (kh,0)+(kh,1) fuse into one K=128
    matmul and taps (kh,2) run zero-padded to K=128 (full-width matmuls
    hold the PE boost clock; K=64 decays to mid p-state).
  - conv results stay in SBUF (no DRAM scratch roundtrip).
  - gamma: the global sum is split in two halves, each AllReduced as soon
    as its half of q finishes; the first collective absorbs CC-core setup
    + cross-core skew so the second (which gates the epilogue) is short.
  - epilogue (exp + add + store) is interleaved per block with the conv
    matmul groups: ACT does exp, DVE drains conv PSUM, GPSIMD+DVE split
    the final add, so the post-PE tail is only a few microseconds.
"""

import os
import sys

import numpy as np

for _p in ("/opt/trn_rl_repo", "/root/.axon_site/_ro/trn_rl_repo"):
    if os.path.isdir(_p) and _p not in sys.path:
        sys.path.insert(0, _p)

import ml_dtypes

import concourse.bacc as bacc
import concourse.bass_utils as _bu
import concourse.tile as tile
from concourse import mybir
from concourse.ap import AP
from concourse.bass_utils import run_bass_kernel_spmd


def _ensure_ntff_hook():
    """bass_utils imports antenv.axon_hooks when tracing under axon; this
    image's antenv lacks that module. Provide it and install the ctypes
    NTFF hook so BASS_TRACE=1 yields exec_time_ns."""
    import types
    try:
        from antenv.axon_hooks import get_axon_ntff_profile_hook  # noqa: F401
        return
    except ImportError:
        pass
    try:
        import antenv
        mod = types.ModuleType("antenv.axon_hooks")
        _state = {"hook": None}
        mod.set_axon_ntff_profile_hook = lambda h: _state.__setitem__("hook", h)
        mod.get_axon_ntff_profile_hook = lambda: _state["hook"]
        sys.modules["antenv.axon_hooks"] = mod
        antenv.axon_hooks = mod
        try:
            from trn_agent_boot.trn_boot import _ntff_profile_via_ctypes
            so = "/opt/axon/libaxon_pjrt.so"
            if os.path.exists(so):
                mod.set_axon_ntff_profile_hook(_ntff_profile_via_ctypes(so))
        except Exception:
            pass
    except Exception:
        pass


# NOTE: walrus's ldw-elision pass (--enable-ldw-opt=true) rejects DoubleRow
# LDWEIGHTS ("InstLdweights is not compatible with LDW optimization"), so
# unlike the fp32r baseline we leave it off: bf16 LDWs get FWL (4-elem-wide
# loads) and shadow-load behind the previous matmul, so elision isn't needed.

_ensure_ntff_hook()

B, H, W, C, F = 32, 64, 64, 64, 128
KH = KW = 3
N_CORES = 8
IMGS = B // N_CORES          # 4 images per core
HP, WP = H + 2, W + 3        # 66 rows, 67 cols (one spare zero col)
ROWS_PER_BLK = 8
BLK = ROWS_PER_BLK * W       # 512 pixels per block
BLKS_PER_IMG = H // ROWS_PER_BLK    # 8
NBLK = IMGS * BLKS_PER_IMG   # 32 blocks per core
PIX = IMGS * H * W           # 16384 pixels per core
NTOT = B * H * W             # 131072 pixels total

F32 = mybir.dt.float32
BF16 = mybir.dt.bfloat16
FP8 = mybir.dt.float8e4
NP_BF16 = ml_dtypes.bfloat16
NP_FP8 = ml_dtypes.float8_e4m3

# q-branch fp8 tile per image: [128, 3 copies (kw shift), HQ rows, 64]
# with contiguous rows; copy c holds x[..., w+c]. HQ=67 adds a zero pad
# row so the lone-tap DoubleRow's dummy second read stays in bounds.
HQ = 67
Q_CS = HQ * W                # copy stride in elements
# DoubleRow tap pairs: (base tap, second tap or None); base tap (kh,kw)
# reads copy kw at row offset kh, the pair stride D walks to the second.
Q_PAIRS = [((0, 0), (0, 1)), ((1, 0), (0, 2)), ((1, 1), (1, 2)),
           ((2, 0), (2, 1)), ((2, 2), None)]
Q_DELTA = [Q_CS, 2 * Q_CS - W, Q_CS, Q_CS, W]
GROUPS = [(0, 1, 2, 3), (4, 5, 6, 7)]   # hb groups within an image
DR = mybir.MatmulPerfMode.DoubleRow

# epilogue add split: blocks [0, STT_SPLIT) on gpsimd, rest on DVE
STT_SPLIT = 22

LAST_EXEC_TIME_NS = None


def _dr_rhs(xt, h0, p):
    """rhs AP [128, 2, 512] for DoubleRow pair p: base tap's 8x64 window is
    one contiguous 512-run; dim1 walks to the second tap via Q_DELTA."""
    (akh, akw), _ = Q_PAIRS[p]
    base = xt[:, akw, h0 + akh:h0 + akh + ROWS_PER_BLK, 0:W]
    raw = base.ap
    part = raw[0]
    new = [part, [Q_DELTA[p], 2], [1, ROWS_PER_BLK * W]]
    return AP(base.tensor, base.offset, new)


def _build(offset_const: float, scale_const: float, n_cores: int = N_CORES):
    """offset_const = 2*sum(cn)/F ; scale_const = -4/(NTOT*F).
    gamma = 1 / (offset_const + scale_const * sum_q_total)."""
    nc = bacc.Bacc("TRN2", target_bir_lowering=False, debug=False,
                   num_devices=n_cores)
    xx = nc.dram_tensor("xx", [128, IMGS, 3, HQ, W], FP8, kind="ExternalInput")
    xc = nc.dram_tensor("xc", [128, IMGS, HP, WP], BF16, kind="ExternalInput")
    qw = nc.dram_tensor("qw", [128, 5, 2, F], FP8, kind="ExternalInput")
    cwp = nc.dram_tensor("cwp", [128, 3, F], BF16, kind="ExternalInput")
    cws = nc.dram_tensor("cws", [128, 3, F], BF16, kind="ExternalInput")
    cnneg = nc.dram_tensor("cnneg", [128, 1], F32, kind="ExternalInput")
    biasf = nc.dram_tensor("biasf", [128, 1], F32, kind="ExternalInput")
    out = nc.dram_tensor("out", [128, PIX], F32, kind="ExternalOutput")

    with tile.TileContext(nc) as tc:
        with (
            tc.tile_pool(name="xp", bufs=1) as xp,
            tc.tile_pool(name="wp", bufs=1) as wp,
            tc.tile_pool(name="qs", bufs=1) as qs,
            tc.tile_pool(name="cs", bufs=24) as cs,
            tc.tile_pool(name="kn", bufs=8) as kn,
            tc.tile_pool(name="ot", bufs=6) as ot,
            tc.tile_pool(name="ps", bufs=6, space="PSUM") as ps,
            tc.tile_pool(name="pss", bufs=1, space="PSUM") as pss,
            tc.tile_pool(name="dr", bufs=1, space="DRAM") as drp,
        ):
            # ---- loads (q weights + first fp8 image first: PE starts asap)
            qwt = wp.tile([128, 5, 2, F], FP8, tag="qw")
            nc.sync.dma_start(out=qwt, in_=qw[:])
            x8 = []
            xcb = []
            for i in range(IMGS):
                t8 = xp.tile([128, 3, HQ, W], FP8, tag=f"x8_{i}")
                nc.sync.dma_start(out=t8, in_=xx[:, i])
                x8.append(t8)
                tb = xp.tile([128, HP, WP], BF16, tag=f"xc_{i}")
                xcb.append(tb)
            cwpt = wp.tile([128, 3, F], BF16, tag="cwp")
            nc.sync.dma_start(out=cwpt, in_=cwp[:])
            cwst = wp.tile([128, 3, F], BF16, tag="cws")
            nc.sync.dma_start(out=cwst, in_=cws[:])
            for i in range(IMGS):
                nc.sync.dma_start(out=xcb[i], in_=xc[:, i])
            cnn = wp.tile([128, 1], F32, tag="cnn")
            nc.sync.dma_start(out=cnn, in_=cnneg[:])
            bft = wp.tile([128, 1], F32, tag="bf")
            nc.sync.dma_start(out=bft, in_=biasf[:])
            ones_c = wp.tile([128, 1], F32, tag="oc")
            nc.vector.memset(ones_c, 1.0)

            qst = qs.tile([128, NBLK, BLK], BF16, tag="q")
            sq_slots = wp.tile([128, NBLK], F32, tag="sq")

            # ---- phase A: q = pc - pn/2, fp8 DoubleRow, 5 matmuls/block
            def q_group(img, grp):
                xt = x8[img]
                qps = [ps.tile([128, BLK], F32, tag="mm", name=f"qp{img}_{hb}")
                       for hb in grp]
                for p in range(len(Q_PAIRS)):
                    wtile = qwt[:, p]
                    for gi, hb in enumerate(grp):
                        rhs = _dr_rhs(xt, hb * ROWS_PER_BLK, p)
                        nc.tensor.matmul(qps[gi][:], wtile, rhs,
                                         start=(p == 0), stop=(p == 4),
                                         perf_mode=DR)
                for gi, hb in enumerate(grp):
                    blk = img * BLKS_PER_IMG + hb
                    nc.scalar.activation(
                        qst[:, blk, :], qps[gi][:],
                        mybir.ActivationFunctionType.Copy,
                        accum_out=sq_slots[:, blk:blk + 1],
                    )

            for img in (0, 1):
                for grp in GROUPS:
                    q_group(img, grp)

            # first-half sum -> CC1 (absorbs CC setup + cross-core skew)
            sq_red_a = wp.tile([128, 1], F32, tag="sqa")
            nc.vector.reduce_sum(sq_red_a, sq_slots[:, 0:16],
                                 axis=mybir.AxisListType.X)
            q_group(2, GROUPS[0])
            ps1a = pss.tile([1, 1], F32, tag="s1a")
            nc.tensor.matmul(ps1a[:], sq_red_a[:], ones_c[:],
                             start=True, stop=True)
            s_a = wp.tile([1, 1], F32, tag="ssa")
            nc.scalar.copy(s_a[:], ps1a[:])
            cc_in_a = drp.tile([1, 1], F32, tag="cia")
            cc_out_a = drp.tile([1, 1], F32, tag="coa")
            nc.sync.dma_start(out=cc_in_a, in_=s_a[:])
            nc.gpsimd.collective_compute(
                "AllReduce", mybir.AluOpType.add,
                replica_groups=[list(range(n_cores))],
                ins=[cc_in_a.opt()], outs=[cc_out_a.opt()],
            )

            q_group(2, GROUPS[1])
            for grp in GROUPS:
                q_group(3, grp)

            sq_red_b = wp.tile([128, 1], F32, tag="sqb")
            nc.vector.reduce_sum(sq_red_b, sq_slots[:, 16:32],
                                 axis=mybir.AxisListType.X)

            # ---- phase C+D: conv (bf16, K=128 pairs) + interleaved epilogue
            scal = wp.tile([128, 1], F32, tag="scal")
            bias_g = wp.tile([128, 1], F32, tag="bg")
            gam128 = wp.tile([128, 1], F32, tag="g128")

            cc_in_b = drp.tile([1, 1], F32, tag="cib")
            cc_out_b = drp.tile([1, 1], F32, tag="cob")

            def conv_group(img, grp):
                xt = xcb[img]
                cps = [ps.tile([128, BLK], F32, tag="mm", name=f"cp{img}_{hb}")
                       for hb in grp]
                for m in range(6):
                    if m < 3:
                        kh, c0, wtile = m, 0, cwpt[:, m]
                    else:
                        kh, c0, wtile = m - 3, 2, cwst[:, m - 3]
                    for gi, hb in enumerate(grp):
                        h0 = hb * ROWS_PER_BLK
                        rhs = xt[:, h0 + kh:h0 + kh + ROWS_PER_BLK, c0:c0 + W]
                        nc.tensor.matmul(cps[gi][:], wtile, rhs,
                                         start=(m == 0), stop=(m == 5))
                return cps

            def drain_block(cps_tile, img, hb):
                # drain conv PSUM and fold in the conv bias, so the final
                # add is a plain tensor_tensor (Pool engine can't take an
                # AP scalar operand)
                blk = img * BLKS_PER_IMG + hb
                cst = cs.tile([128, BLK], F32, tag="cst", name=f"cst{blk}")
                nc.vector.tensor_scalar(
                    out=cst[:], in0=cps_tile[:], scalar1=bft[:], scalar2=None,
                    op0=mybir.AluOpType.add)
                return blk, cst

            def ep_block(blk, cst, pend):
                kant = kn.tile([128, BLK], BF16, tag="kan", name=f"kan{blk}")
                nc.scalar.activation(
                    kant[:], qst[:, blk, :],
                    mybir.ActivationFunctionType.Exp,
                    bias=bias_g[:], scale=scal[:],
                )
                if blk < STT_SPLIT:
                    outt = ot.tile([128, BLK], F32, tag="outt",
                                   name=f"out{blk}")
                    nc.gpsimd.tensor_tensor(
                        out=outt[:], in0=kant[:], in1=cst[:],
                        op=mybir.AluOpType.add,
                    )
                    nc.sync.dma_start(out=out[:, blk * BLK:(blk + 1) * BLK],
                                      in_=outt[:])
                else:
                    pend.append((blk, cst, kant))

            buffered = []      # (blk, cst) drained before gamma is known
            pend = []          # (blk, cst, kant) for DVE-side stts, last
            first = True
            gamma_done = False
            all_groups = [(img, grp) for img in range(IMGS)
                          for grp in GROUPS]
            for gidx, (img, grp) in enumerate(all_groups):
                cps = conv_group(img, grp)
                if first:
                    # second-half sum -> CC2 (PE already busy on conv)
                    ps1b = pss.tile([1, 1], F32, tag="s1b")
                    nc.tensor.matmul(ps1b[:], sq_red_b[:], ones_c[:],
                                     start=True, stop=True)
                    s_b = wp.tile([1, 1], F32, tag="ssb")
                    nc.scalar.copy(s_b[:], ps1b[:])
                    nc.sync.dma_start(out=cc_in_b, in_=s_b[:])
                    nc.gpsimd.collective_compute(
                        "AllReduce", mybir.AluOpType.add,
                        replica_groups=[list(range(n_cores))],
                        ins=[cc_in_b.opt()], outs=[cc_out_b.opt()],
                    )
                    stot_a = wp.tile([1, 1], F32, tag="sta")
                    nc.sync.dma_start(out=stot_a, in_=cc_out_a)
                    stot_b = wp.tile([1, 1], F32, tag="stb")
                    nc.sync.dma_start(out=stot_b, in_=cc_out_b)
                    first = False
                drains = [drain_block(cps[gi], img, hb)
                          for gi, hb in enumerate(grp)]
                if not gamma_done:
                    buffered.extend(drains)
                    if gidx == 4:
                        # gamma chain: emitted only now so every consumer
                        # (ACT exp, stts) is emitted after these writes;
                        # the DVE ops land after drain b19 in DVE order, by
                        # which time CC2's result has long arrived
                        sum2 = wp.tile([1, 1], F32, tag="sum2")
                        nc.vector.tensor_tensor(
                            out=sum2[:], in0=stot_a[:], in1=stot_b[:],
                            op=mybir.AluOpType.add)
                        den = wp.tile([1, 1], F32, tag="den")
                        nc.vector.tensor_scalar(
                            out=den[:], in0=sum2[:],
                            scalar1=float(scale_const),
                            scalar2=float(offset_const),
                            op0=mybir.AluOpType.mult, op1=mybir.AluOpType.add,
                        )
                        gam = wp.tile([1, 1], F32, tag="gam")
                        nc.vector.reciprocal(gam[:], den[:])
                        nc.gpsimd.partition_broadcast(gam128[:], gam[:])
                        nc.vector.tensor_scalar_mul(scal[:], gam128[:], 2.0)
                        nc.vector.tensor_mul(bias_g[:], gam128[:], cnn[:])
                        gamma_done = True
                        for blk, cst in buffered:
                            ep_block(blk, cst, pend)
                else:
                    for blk, cst in drains:
                        ep_block(blk, cst, pend)

            for blk, cst, kant in pend:
                outt = ot.tile([128, BLK], F32, tag="outt", name=f"out{blk}")
                nc.vector.tensor_tensor(
                    out=outt[:], in0=kant[:], in1=cst[:],
                    op=mybir.AluOpType.add,
                )
                nc.sync.dma_start(out=out[:, blk * BLK:(blk + 1) * BLK],
                                  in_=outt[:])

    nc.compile()
    return nc


def _prep_inputs(inputs, kernel, bias, control_points):
    x = np.ascontiguousarray(np.asarray(inputs, dtype=np.float32))
    kw_ = np.asarray(kernel, dtype=np.float32)
    bias = np.asarray(bias, dtype=np.float32)
    cp = np.asarray(control_points, dtype=np.float32)

    # q weights: DoubleRow pairs [c, pair, i, f]; rows 64..127 hit x^2
    qw = np.zeros((128, 5, 2, F), dtype=NP_FP8)
    for p, (a, b) in enumerate(Q_PAIRS):
        for i, t in enumerate((a, b)):
            if t is None:
                continue
            qw[0:C, p, i, :] = cp[t[0], t[1]].astype(NP_FP8)
            qw[C:128, p, i, :] = NP_FP8(-0.5)

    # conv weights: pairs [(kh,0);(kh,1)] and zero-padded singles [(kh,2);0]
    cwp = np.zeros((128, 3, F), dtype=NP_BF16)
    cws = np.zeros((128, 3, F), dtype=NP_BF16)
    for kh in range(KH):
        cwp[0:C, kh, :] = kw_[kh, 0].astype(NP_BF16)
        cwp[C:128, kh, :] = kw_[kh, 1].astype(NP_BF16)
        cws[0:C, kh, :] = kw_[kh, 2].astype(NP_BF16)

    cn = (cp.reshape(KH * KW * C, F).astype(np.float64) ** 2).sum(axis=0)
    offset_const = float(2.0 * cn.sum() / F)
    scale_const = float(-4.0 / (NTOT * F))
    cnneg = np.ascontiguousarray(-cn.astype(np.float32).reshape(F, 1))
    biasf = np.ascontiguousarray(bias.reshape(F, 1))

    in_maps = []
    for core in range(N_CORES):
        xs = x[core * IMGS:(core + 1) * IMGS]          # [4,64,64,64]
        xt = xs.transpose(3, 0, 1, 2)                  # [C,4,64,64]
        xpad = np.zeros((C, IMGS, HP, WP), np.float32)
        xpad[:, :, 1:H + 1, 1:W + 1] = xt
        # fp8 [x | x^2], three column-shifted copies with row stride W
        xx8 = np.zeros((128, IMGS, 3, HQ, W), dtype=NP_FP8)
        xsq = xpad * xpad
        for kwi in range(3):
            sl = xpad[:, :, :, kwi:kwi + W]          # [C, IMGS, HP, W]
            sq = xsq[:, :, :, kwi:kwi + W]
            xx8[0:C, :, kwi, 0:HP, :] = sl.transpose(0, 1, 2, 3).astype(NP_FP8)
            xx8[C:128, :, kwi, 0:HP, :] = sq.astype(NP_FP8)
        # bf16 [x | x shifted left one column]
        xcb = np.zeros((128, IMGS, HP, WP), dtype=NP_BF16)
        xcb[0:C] = xpad.astype(NP_BF16)
        xcb[C:128, :, :, 0:WP - 1] = xpad[:, :, :, 1:WP].astype(NP_BF16)
        in_maps.append({
            "xx": np.ascontiguousarray(xx8),
            "xc": np.ascontiguousarray(xcb),
            "qw": qw, "cwp": cwp, "cws": cws,
            "cnneg": cnneg, "biasf": biasf,
        })
    return in_maps, offset_const, scale_const


def kernel(inputs, kernel, bias, control_points):
    global LAST_EXEC_TIME_NS
    in_maps, offset_const, scale_const = _prep_inputs(
        inputs, kernel, bias, control_points)

    nc = _build(offset_const, scale_const)
    res = run_bass_kernel_spmd(nc, in_maps, core_ids=list(range(N_CORES)))
    LAST_EXEC_TIME_NS = res.exec_time_ns

    out = np.empty((B, H, W, F), np.float32)
    for core in range(N_CORES):
        o = res.results[core]["out"]                   # [128, PIX]
        o = o.reshape(F, IMGS, H, W).transpose(1, 2, 3, 0)
        out[core * IMGS:(core + 1) * IMGS] = o
    return out
